# revision 21
# baseline (speedup 1.0000x reference)
"""Inverted-dropout kernel for Trainium2, distributed over 8 NeuronCores.

Computes out = where(mask, x * 2.0, 0) for x:(64,2048,4,7,7) f32 and
mask:(64,2048,4,7,7) bool.  Pure elementwise: shard along batch (8 per core).

Design (each refinement HW-measured):
- bf16 data path: x is rounded to bf16 on host (~2e-3 rel err, inside the
  2e-2 gate) and the output is stored as bf16, halving both the x read and
  the out write at the HBM.
- Nibble-packed mask: FOUR mask elements per u16 word ({0,2} per nibble,
  the dropout scale folded in), quartering mask HBM traffic.  On device
  each tile's mask unpacks into a u16 {0,2} tile via four two-scalar
  bitwise tensor_scalar ops
      mtb[:, j*q:(j+1)*q] = (mp >> 4j) & 0xF        (q = w/4)
  which run in the DVE 4x_2p fast mode (2-byte in/out, all SBUF), then one
  all-2-byte tensor_mul applies it at 2x_1p.  Net DVE cost ~0.78 ns/elem
  vs 1.04 for a plain u8-mask multiply, and the verifier's "no mixed
  bitwise+arith ops in one instruction" rule is respected.
- Engine split: a slice of tiles goes to the Pool/GPSIMD engine (software
  Q7, ~3.4 ns/elem) so the DVE span stays below the DMA load phase.
- Phase structure: ALL loads enqueue first, then in-place muls, then ALL
  stores.  HWDGE rings drain FIFO per issuing engine, so reads and writes
  phase-separate at the HBM (measured faster than mixed traffic).
- Ramp tiling (6x3584 + 4x896): small tiles last shrink the exposed
  final-tile compute latency between the load and store phases.
- 1D flat DRAM layout: every tile is one fully contiguous chunk viewed as
  [128, w] — max-efficiency DMA descriptors and zero-copy host reshapes.
- Whole per-core shard stays SBUF-resident (in-place output, ~63 KB of the
  208 KB usable per partition).
- Loads/stores alternate between the two HWDGE rings (SP / ACT).
"""

import sys

import numpy as np

try:
    import concourse.bacc as bacc
except ImportError:  # grading env without the default sys.path site config
    for p in ("/root/.axon_site/_ro/trn_rl_repo", "/opt/trn_rl_repo"):
        if p not in sys.path:
            sys.path.append(p)
    import concourse.bacc as bacc

import concourse.mybir as mybir
from concourse.tile import TileContext

# Full problem shape (hardcoded per harness contract).
B, C, FM, H, W = 64, 2048, 4, 7, 7
N_CORES = 8
B_PER_CORE = B // N_CORES                       # 8
ELEMS_PER_CORE = B_PER_CORE * C * FM * H * W    # 3,211,264 = 128 * 25088

P = 128                                         # SBUF partitions
TOTAL_F = ELEMS_PER_CORE // P                   # 25088 free-dim elems/partition
SIZES = [3584] * 6 + [896] * 4                  # ramp: small tiles last
MASK_GROUPS = [4, 6]                            # mask loads: tiles 0-3, 4-9
# Multiply engine per tile: 'v' = DVE, 'p' = Pool/GPSIMD.  The bitwise
# unpack is DVE-only (TensorScalarPtr is illegal on Pool); DVE unpacks all
# tiles (~0.26 ns/elem at 4x) and muls its own at 2x (~0.52), while Pool
# (~2 ns/elem) muls two early big tiles to keep DVE under the load phase.
ENGS = ['p', 'p', 'v', 'v', 'v', 'v', 'v', 'v', 'v', 'v']
assert sum(SIZES) == TOTAL_F

SCALE = 2.0      # 1 / (1 - p_drop), p_drop = 0.5


def build_nc(sizes=None, mask_groups=None, engs=None, repeat=1,
             rev_store=False, load_qs=("sync", "scalar"),
             store_qs=("scalar", "sync"), mask_bits=False):
    """Build the per-core SPMD module (phase-structured, ramp-tiled).

    Bacc (not bare Bass): Bacc.compile() legalizes sync waits down to the
    TRN2 1-wait-per-instruction limit — walrus rejects the module otherwise.

    repeat>1 unrolls the whole body R times inside one NEFF (idempotent
    rewrites of the same output), used only for launch-overhead-free timing
    via (T(R2)-T(R1))/(R2-R1).  rev_store reverses per-repeat store order so
    cross-repeat WAR chains approximate clean serial load/store phases
    (timing only; production single-shot uses forward order).
    """
    sizes = sizes or SIZES
    mask_groups = mask_groups or MASK_GROUPS
    engs = engs or ENGS
    if isinstance(engs, str):
        engs = list(engs)
    n = P * sum(sizes)
    nc = bacc.Bacc()
    # mask_bits: 16 mask bits per u16 word, {0,1} values, x pre-scaled by 2
    # on host.  Default: 4 nibbles per u16 word, {0,2} values.
    mdiv = 16 if mask_bits else 4
    x = nc.declare_dram_parameter("x", [n], mybir.dt.bfloat16, isOutput=False)
    m = nc.declare_dram_parameter("mask", [n // mdiv], mybir.dt.uint16,
                                  isOutput=False)
    o = nc.declare_dram_parameter("out", [n], mybir.dt.bfloat16, isOutput=True)
    offs = np.cumsum([0] + list(sizes))[:-1]
    gb = np.cumsum([0] + list(mask_groups))
    granges = [
        (offs[gb[k]], (offs[gb[k + 1] - 1] + sizes[gb[k + 1] - 1]) - offs[gb[k]])
        for k in range(len(mask_groups))
    ]
    tile2group = {t: k for k in range(len(mask_groups))
                  for t in range(gb[k], gb[k + 1])}

    def sl(t, a, w):
        # contiguous flat chunk [128*a, 128*(a+w)) viewed as [128, w]
        return t[P * a: P * (a + w)].rearrange("(p w) -> p w", p=P)

    with TileContext(nc) as tc:
        with tc.tile_pool(name="sbuf", bufs=1) as pool:
            for _ in range(repeat):
                # nibble-packed mask first, as big merged transfers (host
                # emits the mask operand in matching group-major layout)
                mtiles = []
                for k, (ga, gw) in enumerate(granges):
                    eng = getattr(nc, load_qs[k % len(load_qs)])
                    mt = pool.tile([P, gw // mdiv], mybir.dt.uint16,
                                   tag=f"mt{k}")
                    eng.dma_start(out=mt[:], in_=sl(m, ga // mdiv, gw // mdiv))
                    mtiles.append((mt, ga))
                xts = []
                for i, (a, w) in enumerate(zip(offs, sizes)):
                    load_eng = getattr(nc, load_qs[i % len(load_qs)])
                    xt = pool.tile([P, w], mybir.dt.bfloat16, tag=f"xt{i}")
                    load_eng.dma_start(out=xt[:], in_=sl(x, a, w))
                    xts.append(xt)
                # unpack each mask group to a full-width u16 multiplier tile
                # ({0,2} nibbles, or {0,1} bits with x pre-scaled by 2):
                # field j of word i covers group column j*gq + i
                mubs = []
                shift, fmask = (1, 0x1) if mask_bits else (4, 0xF)
                for k, (ga, gw) in enumerate(granges):
                    mt, _ = mtiles[k]
                    gq = gw // mdiv
                    mub = pool.tile([P, gw], mybir.dt.uint16, tag=f"mu{k}")
                    for j in range(mdiv):
                        nc.vector.tensor_scalar(
                            out=mub[:, j * gq:(j + 1) * gq], in0=mt[:],
                            scalar1=shift * j, scalar2=fmask,
                            op0=mybir.AluOpType.logical_shift_right,
                            op1=mybir.AluOpType.bitwise_and)
                    mubs.append(mub)
                for i, (a, w) in enumerate(zip(offs, sizes)):
                    k = tile2group[i]
                    ga = granges[k][0]
                    ce = nc.vector if engs[i] == 'v' else nc.gpsimd
                    ce.tensor_mul(out=xts[i][:], in0=xts[i][:],
                                  in1=mubs[k][:, a - ga: a - ga + w])
                order = reversed(range(len(sizes))) if rev_store \
                    else range(len(sizes))
                for i in order:
                    store_eng = getattr(nc, store_qs[i % len(store_qs)])
                    store_eng.dma_start(
                        out=sl(o, offs[i], sizes[i]), in_=xts[i][:])
    nc.compile()
    return nc


def _mask_layout(mflat_global, mask_bits=False, sizes=None, mask_groups=None):
    """Host staging of the mask operand: from the (N_CORES*ELEMS,) flat
    {0,1} byte mask, produce the packed group-major u16 words build_nc's
    merged group loads expect.

    Per group [P, gw]: word i of the packed [P, gw/mdiv] group tile holds
    group column j*(gw/mdiv) + i in field j — value m*2 per nibble
    (mdiv=4), or m per bit (mdiv=16, x pre-scaled by 2 on host).
    """
    sizes = sizes or SIZES
    mask_groups = mask_groups or MASK_GROUPS
    mdiv, shift = (16, 1) if mask_bits else (4, 4)
    pre = 0 if mask_bits else 1           # {0,1} bits vs {0,2} nibbles
    offs = np.cumsum([0] + list(sizes))[:-1]
    gb = np.cumsum([0] + list(mask_groups))
    per_core = mflat_global.reshape(N_CORES, ELEMS_PER_CORE)
    out = np.empty((N_CORES, ELEMS_PER_CORE // mdiv), dtype=np.uint16)
    for c in range(N_CORES):
        t2d = [per_core[c, P * a: P * (a + w)].reshape(P, w)
               for a, w in zip(offs, sizes)]
        pos = 0
        for k in range(len(mask_groups)):
            g = np.concatenate(t2d[gb[k]:gb[k + 1]], axis=1).astype(np.uint16)
            gq = g.shape[1] // mdiv
            g3 = g.reshape(P, mdiv, gq)
            words = np.zeros((P, gq), dtype=np.uint16)
            for j in range(mdiv):
                words |= g3[:, j, :] << (shift * j + pre)
            w_flat = np.ascontiguousarray(words).ravel()
            out[c, pos:pos + w_flat.size] = w_flat
            pos += w_flat.size
    return out.reshape(-1)


def _build_runner(nc, n_cores):
    """Compile the SPMD module into a reusable shard_map-jitted callable.

    Same machinery as bass2jax.run_bass_via_pjrt, but the jitted function is
    built once and cached so repeated kernel() calls skip XLA re-tracing.
    Output-buffer donation is dropped: this kernel writes every output
    element, so zero-initialized outputs are unnecessary.
    """
    import jax
    from jax.sharding import Mesh, PartitionSpec, NamedSharding
    from jax.experimental.shard_map import shard_map
    from concourse.bass2jax import (
        _bass_exec_p,
        install_neuronx_cc_hook,
        partition_id_tensor,
    )

    install_neuronx_cc_hook()
    partition_name = nc.partition_id_tensor.name if nc.partition_id_tensor else None

    in_names, out_names, out_avals = [], [], []
    for alloc in nc.m.functions[0].allocations:
        if not isinstance(alloc, mybir.MemoryLocationSet):
            continue
        name = alloc.memorylocations[0].name
        if alloc.kind == "ExternalInput":
            if name != partition_name:
                in_names.append(name)
        elif alloc.kind == "ExternalOutput":
            out_names.append(name)
            out_avals.append(
                jax.core.ShapedArray(
                    tuple(alloc.tensor_shape), mybir.dt.np(alloc.dtype)
                )
            )
    n_params = len(in_names)
    all_in_names = list(in_names) + list(out_names)
    if partition_name is not None:
        all_in_names.append(partition_name)

    def _body(*args):
        operands = list(args)
        if partition_name is not None:
            operands.append(partition_id_tensor())
        outs = _bass_exec_p.bind(
            *operands,
            out_avals=tuple(out_avals),
            in_names=tuple(all_in_names),
            out_names=tuple(out_names),
            lowering_input_output_aliases=(),
            sim_require_finite=True,
            sim_require_nnan=True,
            nc=nc,
        )
        return tuple(outs)

    devices = jax.devices()[:n_cores]
    assert len(devices) == n_cores, (
        f"need {n_cores} devices, have {len(jax.devices())}"
    )
    mesh = Mesh(np.asarray(devices), ("core",))
    in_specs = (PartitionSpec("core"),) * (n_params + len(out_names))
    out_specs = (PartitionSpec("core"),) * len(out_names)
    fn = jax.jit(
        shard_map(
            _body, mesh=mesh, in_specs=in_specs, out_specs=out_specs,
            check_rep=False,
        ),
        keep_unused=True,
    )
    sharding = NamedSharding(mesh, PartitionSpec("core"))
    zeros = [
        np.zeros((n_cores * a.shape[0], *a.shape[1:]), a.dtype) for a in out_avals
    ]
    return fn, sharding, in_names, out_avals, zeros


_CACHE = {}


def _get_runner():
    if "runner" not in _CACHE:
        nc = build_nc()
        _CACHE["runner"] = _build_runner(nc, N_CORES)
    return _CACHE["runner"]


def kernel(x: np.ndarray, mask: np.ndarray, **_) -> np.ndarray:
    import jax
    import ml_dtypes

    x = np.ascontiguousarray(np.asarray(x), dtype=np.float32)
    mask = np.asarray(mask)
    if mask.dtype.itemsize != 1:
        mask = mask.astype(np.bool_)
    mask = np.ascontiguousarray(mask)
    assert x.shape == (B, C, FM, H, W), x.shape
    assert mask.shape == (B, C, FM, H, W), mask.shape

    fn, sharding, in_names, out_avals, zeros = _get_runner()
    # Flat layout: batch-sharding == contiguous row-blocks, and the kernel's
    # element order is plain C order, so x is a zero-copy reshape.  x
    # travels as bf16 (round-to-nearest); the mask travels nibble-packed
    # with the 1/(1-p)=2.0 dropout scale folded into the nibble values.
    global_in = {
        "x": x.astype(ml_dtypes.bfloat16).reshape(N_CORES * ELEMS_PER_CORE),
        "mask": _mask_layout(
            mask.view(np.uint8).reshape(N_CORES * ELEMS_PER_CORE)),
    }
    if "zeros_dev" not in _CACHE:
        # Output buffers are fully overwritten by the kernel; stage once and
        # reuse across calls (not donated).
        _CACHE["zeros_dev"] = [jax.device_put(z, sharding) for z in zeros]
    args = [jax.device_put(global_in[n], sharding) for n in in_names]
    args += _CACHE["zeros_dev"]
    out = jax.block_until_ready(fn(*args))
    return np.asarray(out[0]).astype(np.float32).reshape(B, C, FM, H, W)


# revision 32
# speedup vs baseline: 1.1356x; 1.1356x over previous
"""Inverted-dropout kernel for Trainium2, distributed over 8 NeuronCores.

Computes out = where(mask, x * 2.0, 0) for x:(64,2048,4,7,7) f32 and
mask:(64,2048,4,7,7) bool.  Pure elementwise: shard along batch (8 per core).

Design (each refinement HW-measured; the kernel is per-core DMA-bandwidth
bound at ~330-450 GB/s effective, so bytes moved dominate):
- bf16 data path: host scales x by 2 (the 1/(1-p) dropout factor, exact in
  bf16) and rounds to bf16 (~4e-3 rel err, inside the 2e-2 gate); the
  output is stored as bf16 and widened to f32 on host.  Halves both the x
  read and the out write vs f32.
- Bit-packed mask: SIXTEEN mask elements per u16 word, cutting mask HBM
  traffic 16x (0.20 MB/core).  On device each mask group unpacks into a
  u16 {0,1} multiplier tile via sixteen two-scalar bitwise tensor_scalar
  ops   mub[:, j*gq:(j+1)*gq] = (mt >> j) & 1   which run in the DVE
  4x_2p fast mode (2-byte in/out, all SBUF, ~0.26 ns/elem); one
  all-2-byte tensor_mul per tile applies it at 2x_1p (~0.52 ns/elem).
  (A fused unpack+multiply is illegal: the walrus verifier rejects mixed
  bitwise+arith ops in one instruction, and TensorScalarPtr is illegal on
  Pool.)
- Engine split: Pool/GPSIMD (software Q7, ~2 ns/elem tensor_mul) muls two
  early big tiles so the DVE span stays below the DMA load phase;
  Pool-owned tiles store last (pool_last) so the store rings never stall
  on Pool's slower muls.
- Phase structure: ALL loads enqueue first, then in-place muls, then ALL
  stores.  HWDGE rings drain FIFO per issuing engine, so reads and writes
  phase-separate at the HBM (measured faster than mixed traffic).
- Ramp tiling (6x3584 + 4x896): small tiles last shrink the exposed
  final-tile compute latency between the load and store phases.
- 1D flat DRAM layout: every tile is one fully contiguous chunk viewed as
  [128, w] — max-efficiency DMA descriptors and zero-copy host reshapes.
- Whole per-core shard stays SBUF-resident (in-place output, ~103 KB of
  the 208 KB usable per partition).
- Loads/stores alternate between the two HWDGE rings (SP / ACT); a third
  SWDGE channel via gpsimd was measured and does NOT add bandwidth.
"""

import sys

import numpy as np

try:
    import concourse.bacc as bacc
except ImportError:  # grading env without the default sys.path site config
    for p in ("/root/.axon_site/_ro/trn_rl_repo", "/opt/trn_rl_repo"):
        if p not in sys.path:
            sys.path.append(p)
    import concourse.bacc as bacc

import concourse.mybir as mybir
from concourse.tile import TileContext

# Full problem shape (hardcoded per harness contract).
B, C, FM, H, W = 64, 2048, 4, 7, 7
N_CORES = 8
B_PER_CORE = B // N_CORES                       # 8
ELEMS_PER_CORE = B_PER_CORE * C * FM * H * W    # 3,211,264 = 128 * 25088

P = 128                                         # SBUF partitions
TOTAL_F = ELEMS_PER_CORE // P                   # 25088 free-dim elems/partition
SIZES = [3584] * 6 + [896] * 4                  # ramp: small tiles last
MASK_GROUPS = [4, 6]                            # mask loads: tiles 0-3, 4-9
# Multiply engine per tile: 'v' = DVE, 'p' = Pool/GPSIMD.  The bitwise
# unpack is DVE-only (TensorScalarPtr is illegal on Pool); DVE unpacks all
# tiles (~0.26 ns/elem at 4x) and muls its own at 2x (~0.52), while Pool
# (~2 ns/elem) muls two early big tiles to keep DVE under the load phase.
ENGS = ['p', 'p', 'v', 'v', 'v', 'v', 'v', 'v', 'v', 'v']
assert sum(SIZES) == TOTAL_F

MASK_BITS = True    # False: 4 nibbles/u16 word {0,2}; True: 16 bits {0,1}
POOL_LAST = True    # store Pool-computed tiles after DVE tiles

SCALE = 2.0      # 1 / (1 - p_drop), p_drop = 0.5


def build_nc(sizes=None, mask_groups=None, engs=None, repeat=1,
             rev_store=False, load_qs=("sync", "scalar"),
             store_qs=("scalar", "sync"), mask_bits=None, pool_last=None,
             swdge_tiles=()):
    """Build the per-core SPMD module (phase-structured, ramp-tiled).

    Bacc (not bare Bass): Bacc.compile() legalizes sync waits down to the
    TRN2 1-wait-per-instruction limit — walrus rejects the module otherwise.

    repeat>1 unrolls the whole body R times inside one NEFF (idempotent
    rewrites of the same output), used only for launch-overhead-free timing
    via (T(R2)-T(R1))/(R2-R1).  rev_store reverses per-repeat store order so
    cross-repeat WAR chains approximate clean serial load/store phases
    (timing only; production single-shot uses forward order).
    """
    sizes = sizes or SIZES
    mask_groups = mask_groups or MASK_GROUPS
    engs = engs or ENGS
    if isinstance(engs, str):
        engs = list(engs)
    if mask_bits is None:
        mask_bits = MASK_BITS
    if pool_last is None:
        pool_last = POOL_LAST
    n = P * sum(sizes)
    nc = bacc.Bacc()
    # mask_bits: 16 mask bits per u16 word, {0,1} values, x pre-scaled by 2
    # on host.  Default: 4 nibbles per u16 word, {0,2} values.
    mdiv = 16 if mask_bits else 4
    x = nc.declare_dram_parameter("x", [n], mybir.dt.bfloat16, isOutput=False)
    m = nc.declare_dram_parameter("mask", [n // mdiv], mybir.dt.uint16,
                                  isOutput=False)
    o = nc.declare_dram_parameter("out", [n], mybir.dt.bfloat16, isOutput=True)
    offs = np.cumsum([0] + list(sizes))[:-1]
    gb = np.cumsum([0] + list(mask_groups))
    granges = [
        (offs[gb[k]], (offs[gb[k + 1] - 1] + sizes[gb[k + 1] - 1]) - offs[gb[k]])
        for k in range(len(mask_groups))
    ]
    tile2group = {t: k for k in range(len(mask_groups))
                  for t in range(gb[k], gb[k + 1])}

    def sl(t, a, w):
        # contiguous flat chunk [128*a, 128*(a+w)) viewed as [128, w]
        return t[P * a: P * (a + w)].rearrange("(p w) -> p w", p=P)

    with TileContext(nc) as tc:
        with tc.tile_pool(name="sbuf", bufs=1) as pool:
            for _ in range(repeat):
                # nibble-packed mask first, as big merged transfers (host
                # emits the mask operand in matching group-major layout)
                mtiles = []
                for k, (ga, gw) in enumerate(granges):
                    eng = getattr(nc, load_qs[k % len(load_qs)])
                    mt = pool.tile([P, gw // mdiv], mybir.dt.uint16,
                                   tag=f"mt{k}")
                    eng.dma_start(out=mt[:], in_=sl(m, ga // mdiv, gw // mdiv))
                    mtiles.append((mt, ga))
                xts = []
                for i, (a, w) in enumerate(zip(offs, sizes)):
                    if i in swdge_tiles:
                        load_eng = nc.gpsimd       # 3rd channel: SWDGE
                    else:
                        load_eng = getattr(nc, load_qs[i % len(load_qs)])
                    xt = pool.tile([P, w], mybir.dt.bfloat16, tag=f"xt{i}")
                    load_eng.dma_start(out=xt[:], in_=sl(x, a, w))
                    xts.append(xt)
                # unpack each mask group to a full-width u16 multiplier tile
                # ({0,2} nibbles, or {0,1} bits with x pre-scaled by 2):
                # field j of word i covers group column j*gq + i
                mubs = []
                shift, fmask = (1, 0x1) if mask_bits else (4, 0xF)
                for k, (ga, gw) in enumerate(granges):
                    mt, _ = mtiles[k]
                    gq = gw // mdiv
                    mub = pool.tile([P, gw], mybir.dt.uint16, tag=f"mu{k}")
                    for j in range(mdiv):
                        nc.vector.tensor_scalar(
                            out=mub[:, j * gq:(j + 1) * gq], in0=mt[:],
                            scalar1=shift * j, scalar2=fmask,
                            op0=mybir.AluOpType.logical_shift_right,
                            op1=mybir.AluOpType.bitwise_and)
                    mubs.append(mub)
                for i, (a, w) in enumerate(zip(offs, sizes)):
                    k = tile2group[i]
                    ga = granges[k][0]
                    ce = nc.vector if engs[i] == 'v' else nc.gpsimd
                    ce.tensor_mul(out=xts[i][:], in0=xts[i][:],
                                  in1=mubs[k][:, a - ga: a - ga + w])
                order = list(range(len(sizes)))
                if pool_last:
                    # Pool muls finish after the load phase; storing their
                    # tiles last keeps the store rings from stalling on them
                    order = ([i for i in order if engs[i] == 'v']
                             + [i for i in order if engs[i] == 'p'])
                if rev_store:
                    order = order[::-1]
                for pos, i in enumerate(order):
                    if i in swdge_tiles:
                        store_eng = nc.gpsimd
                    else:
                        store_eng = getattr(nc, store_qs[pos % len(store_qs)])
                    store_eng.dma_start(
                        out=sl(o, offs[i], sizes[i]), in_=xts[i][:])
    nc.compile()
    return nc


def _mask_layout(mflat_global, mask_bits=False, sizes=None, mask_groups=None):
    """Host staging of the mask operand: from the (N_CORES*ELEMS,) flat
    {0,1} byte mask, produce the packed group-major u16 words build_nc's
    merged group loads expect.

    Per group [P, gw]: word i of the packed [P, gw/mdiv] group tile holds
    group column j*(gw/mdiv) + i in field j — value m*2 per nibble
    (mdiv=4), or m per bit (mdiv=16, x pre-scaled by 2 on host).
    """
    sizes = sizes or SIZES
    mask_groups = mask_groups or MASK_GROUPS
    mdiv, shift = (16, 1) if mask_bits else (4, 4)
    pre = 0 if mask_bits else 1           # {0,1} bits vs {0,2} nibbles
    offs = np.cumsum([0] + list(sizes))[:-1]
    gb = np.cumsum([0] + list(mask_groups))
    per_core = mflat_global.reshape(N_CORES, ELEMS_PER_CORE)
    out = np.empty((N_CORES, ELEMS_PER_CORE // mdiv), dtype=np.uint16)
    for c in range(N_CORES):
        t2d = [per_core[c, P * a: P * (a + w)].reshape(P, w)
               for a, w in zip(offs, sizes)]
        pos = 0
        for k in range(len(mask_groups)):
            g = np.concatenate(t2d[gb[k]:gb[k + 1]], axis=1).astype(np.uint16)
            gq = g.shape[1] // mdiv
            g3 = g.reshape(P, mdiv, gq)
            words = np.zeros((P, gq), dtype=np.uint16)
            for j in range(mdiv):
                words |= g3[:, j, :] << (shift * j + pre)
            w_flat = np.ascontiguousarray(words).ravel()
            out[c, pos:pos + w_flat.size] = w_flat
            pos += w_flat.size
    return out.reshape(-1)


def _build_runner(nc, n_cores):
    """Compile the SPMD module into a reusable shard_map-jitted callable.

    Same machinery as bass2jax.run_bass_via_pjrt, but the jitted function is
    built once and cached so repeated kernel() calls skip XLA re-tracing.
    Output-buffer donation is dropped: this kernel writes every output
    element, so zero-initialized outputs are unnecessary.
    """
    import jax
    from jax.sharding import Mesh, PartitionSpec, NamedSharding
    from jax.experimental.shard_map import shard_map
    from concourse.bass2jax import (
        _bass_exec_p,
        install_neuronx_cc_hook,
        partition_id_tensor,
    )

    install_neuronx_cc_hook()
    partition_name = nc.partition_id_tensor.name if nc.partition_id_tensor else None

    in_names, out_names, out_avals = [], [], []
    for alloc in nc.m.functions[0].allocations:
        if not isinstance(alloc, mybir.MemoryLocationSet):
            continue
        name = alloc.memorylocations[0].name
        if alloc.kind == "ExternalInput":
            if name != partition_name:
                in_names.append(name)
        elif alloc.kind == "ExternalOutput":
            out_names.append(name)
            out_avals.append(
                jax.core.ShapedArray(
                    tuple(alloc.tensor_shape), mybir.dt.np(alloc.dtype)
                )
            )
    n_params = len(in_names)
    all_in_names = list(in_names) + list(out_names)
    if partition_name is not None:
        all_in_names.append(partition_name)

    def _body(*args):
        operands = list(args)
        if partition_name is not None:
            operands.append(partition_id_tensor())
        outs = _bass_exec_p.bind(
            *operands,
            out_avals=tuple(out_avals),
            in_names=tuple(all_in_names),
            out_names=tuple(out_names),
            lowering_input_output_aliases=(),
            sim_require_finite=True,
            sim_require_nnan=True,
            nc=nc,
        )
        return tuple(outs)

    devices = jax.devices()[:n_cores]
    assert len(devices) == n_cores, (
        f"need {n_cores} devices, have {len(jax.devices())}"
    )
    mesh = Mesh(np.asarray(devices), ("core",))
    in_specs = (PartitionSpec("core"),) * (n_params + len(out_names))
    out_specs = (PartitionSpec("core"),) * len(out_names)
    fn = jax.jit(
        shard_map(
            _body, mesh=mesh, in_specs=in_specs, out_specs=out_specs,
            check_rep=False,
        ),
        keep_unused=True,
    )
    sharding = NamedSharding(mesh, PartitionSpec("core"))
    zeros = [
        np.zeros((n_cores * a.shape[0], *a.shape[1:]), a.dtype) for a in out_avals
    ]
    return fn, sharding, in_names, out_avals, zeros


_CACHE = {}


def _get_runner():
    if "runner" not in _CACHE:
        nc = build_nc()
        _CACHE["runner"] = _build_runner(nc, N_CORES)
    return _CACHE["runner"]


def kernel(x: np.ndarray, mask: np.ndarray, **_) -> np.ndarray:
    import jax
    import ml_dtypes

    x = np.ascontiguousarray(np.asarray(x), dtype=np.float32)
    mask = np.asarray(mask)
    if mask.dtype.itemsize != 1:
        mask = mask.astype(np.bool_)
    mask = np.ascontiguousarray(mask)
    assert x.shape == (B, C, FM, H, W), x.shape
    assert mask.shape == (B, C, FM, H, W), mask.shape

    fn, sharding, in_names, out_avals, zeros = _get_runner()
    # Flat layout: batch-sharding == contiguous row-blocks, and the kernel's
    # element order is plain C order, so x is a zero-copy reshape.  x
    # travels as bf16 (round-to-nearest); the mask travels nibble-packed
    # with the 1/(1-p)=2.0 dropout scale folded into the nibble values.
    xs = x * np.float32(2.0) if MASK_BITS else x   # {0,1} bit masks need 2x
    global_in = {
        "x": xs.astype(ml_dtypes.bfloat16).reshape(N_CORES * ELEMS_PER_CORE),
        "mask": _mask_layout(
            mask.view(np.uint8).reshape(N_CORES * ELEMS_PER_CORE),
            mask_bits=MASK_BITS),
    }
    if "zeros_dev" not in _CACHE:
        # Output buffers are fully overwritten by the kernel; stage once and
        # reuse across calls (not donated).
        _CACHE["zeros_dev"] = [jax.device_put(z, sharding) for z in zeros]
    args = [jax.device_put(global_in[n], sharding) for n in in_names]
    args += _CACHE["zeros_dev"]
    out = jax.block_until_ready(fn(*args))
    return np.asarray(out[0]).astype(np.float32).reshape(B, C, FM, H, W)


# revision 50
# speedup vs baseline: 1.2781x; 1.1255x over previous
"""Inverted-dropout kernel for Trainium2, distributed over 8 NeuronCores.

Computes out = where(mask, x * 2.0, 0) for x:(64,2048,4,7,7) f32 and
mask:(64,2048,4,7,7) bool.  Pure elementwise: shard along batch (8 per core).

Design (each refinement HW-measured; the kernel is per-core DMA-bandwidth
bound at ~330-450 GB/s effective, so bytes moved dominate):
- bf16 data path: host scales x by 2 (the 1/(1-p) dropout factor, exact in
  bf16) and rounds to bf16 (~4e-3 rel err, inside the 2e-2 gate); the
  output is stored as bf16 and widened to f32 on host.  Halves both the x
  read and the out write vs f32.
- Bit-packed mask: SIXTEEN mask elements per u16 word, cutting mask HBM
  traffic 16x (0.20 MB/core).  On device each mask group unpacks into a
  u16 {0,1} multiplier tile via sixteen two-scalar bitwise tensor_scalar
  ops   mub[:, j*gq:(j+1)*gq] = (mt >> j) & 1   which run in the DVE
  4x_2p fast mode (2-byte in/out, all SBUF, ~0.26 ns/elem); one
  all-2-byte tensor_mul per tile applies it at 2x_1p (~0.52 ns/elem).
  (A fused unpack+multiply is illegal: the walrus verifier rejects mixed
  bitwise+arith ops in one instruction, and TensorScalarPtr is illegal on
  Pool.)
- Engine split: Pool/GPSIMD (software Q7, ~2 ns/elem tensor_mul) muls two
  early big tiles so the DVE span stays below the DMA load phase;
  Pool-owned tiles store last (pool_last) so the store rings never stall
  on Pool's slower muls.
- Phase structure: ALL loads enqueue first, then in-place muls, then ALL
  stores.  HWDGE rings drain FIFO per issuing engine, so reads and writes
  phase-separate at the HBM (measured faster than mixed traffic).
- Ramp tiling (6x3584 + 4x896): small tiles last shrink the exposed
  final-tile compute latency between the load and store phases.
- 1D flat DRAM layout: every tile is one fully contiguous chunk viewed as
  [128, w] — max-efficiency DMA descriptors and zero-copy host reshapes.
- Whole per-core shard stays SBUF-resident (in-place output, ~103 KB of
  the 208 KB usable per partition).
- Loads/stores alternate between the two HWDGE rings (SP / ACT); a third
  SWDGE channel via gpsimd was measured and does NOT add bandwidth.
"""

import sys

import numpy as np

try:
    import concourse.bacc as bacc
except ImportError:  # grading env without the default sys.path site config
    for p in ("/root/.axon_site/_ro/trn_rl_repo", "/opt/trn_rl_repo"):
        if p not in sys.path:
            sys.path.append(p)
    import concourse.bacc as bacc

import concourse.mybir as mybir
from concourse.tile import TileContext

# Full problem shape (hardcoded per harness contract).
B, C, FM, H, W = 64, 2048, 4, 7, 7
N_CORES = 8
B_PER_CORE = B // N_CORES                       # 8
ELEMS_PER_CORE = B_PER_CORE * C * FM * H * W    # 3,211,264 = 128 * 25088

P = 128                                         # SBUF partitions
TOTAL_F = ELEMS_PER_CORE // P                   # 25088 free-dim elems/partition
SIZES = [3584] * 6 + [896] * 4                  # ramp: small tiles last
MASK_GROUPS = [4, 6]                            # mask loads: tiles 0-3, 4-9
# Multiply engine per tile: 'v' = DVE, 'p' = Pool/GPSIMD.  The bitwise
# unpack is DVE-only (TensorScalarPtr is illegal on Pool); DVE unpacks all
# tiles (~0.26 ns/elem at 4x) and muls its own at 2x (~0.52), while Pool
# (~2 ns/elem) muls two early big tiles to keep DVE under the load phase.
ENGS = ['p', 'p', 'v', 'v', 'v', 'v', 'v', 'v', 'v', 'v']
assert sum(SIZES) == TOTAL_F

MASK_BITS = True    # False: 4 nibbles/u16 word {0,2}; True: 16 bits {0,1}
POOL_LAST = True    # store Pool-computed tiles after DVE tiles
INTERLEAVE = 0      # 0: serial load/store phases; L>0: mixed traffic with
                    # store lag L (use with engs all-'v')

SCALE = 2.0      # 1 / (1 - p_drop), p_drop = 0.5


def build_nc(sizes=None, mask_groups=None, engs=None, repeat=1,
             rev_store=False, load_qs=("sync", "scalar"),
             store_qs=("scalar", "sync"), mask_bits=None, pool_last=None,
             swdge_tiles=(), mode="full", interleave=None):
    """Build the per-core SPMD module (phase-structured, ramp-tiled).

    Bacc (not bare Bass): Bacc.compile() legalizes sync waits down to the
    TRN2 1-wait-per-instruction limit — walrus rejects the module otherwise.

    repeat>1 unrolls the whole body R times inside one NEFF (idempotent
    rewrites of the same output), used only for launch-overhead-free timing
    via (T(R2)-T(R1))/(R2-R1).  rev_store reverses per-repeat store order so
    cross-repeat WAR chains approximate clean serial load/store phases
    (timing only; production single-shot uses forward order).
    """
    sizes = sizes or SIZES
    mask_groups = mask_groups or MASK_GROUPS
    engs = engs or ENGS
    if isinstance(engs, str):
        engs = list(engs)
    if mask_bits is None:
        mask_bits = MASK_BITS
    if pool_last is None:
        pool_last = POOL_LAST
    if interleave is None:
        interleave = INTERLEAVE
    n = P * sum(sizes)
    nc = bacc.Bacc()
    # mask_bits: 16 mask bits per u16 word, {0,1} values, x pre-scaled by 2
    # on host.  Default: 4 nibbles per u16 word, {0,2} values.
    mdiv = 16 if mask_bits else 4
    x = nc.declare_dram_parameter("x", [n], mybir.dt.bfloat16, isOutput=False)
    m = nc.declare_dram_parameter("mask", [n // mdiv], mybir.dt.uint16,
                                  isOutput=False)
    o = nc.declare_dram_parameter("out", [n], mybir.dt.bfloat16, isOutput=True)
    offs = np.cumsum([0] + list(sizes))[:-1]
    gb = np.cumsum([0] + list(mask_groups))
    granges = [
        (offs[gb[k]], (offs[gb[k + 1] - 1] + sizes[gb[k + 1] - 1]) - offs[gb[k]])
        for k in range(len(mask_groups))
    ]
    tile2group = {t: k for k in range(len(mask_groups))
                  for t in range(gb[k], gb[k + 1])}

    def sl(t, a, w):
        # contiguous flat chunk [128*a, 128*(a+w)) viewed as [128, w]
        return t[P * a: P * (a + w)].rearrange("(p w) -> p w", p=P)

    with TileContext(nc) as tc:
        with tc.tile_pool(name="sbuf", bufs=1) as pool:
            for rep in range(repeat):
                # nibble-packed mask first, as big merged transfers (host
                # emits the mask operand in matching group-major layout)
                mtiles = []
                for k, (ga, gw) in enumerate(granges):
                    eng = getattr(nc, load_qs[k % len(load_qs)])
                    mt = pool.tile([P, gw // mdiv], mybir.dt.uint16,
                                   tag=f"mt{k}")
                    eng.dma_start(out=mt[:],
                                  in_=sl(m, ga // mdiv, gw // mdiv))
                    mtiles.append((mt, ga))
                shift, fmask = (1, 0x1) if mask_bits else (4, 0xF)
                if interleave:
                    # Mixed-traffic schedule: per tile, load_i is followed
                    # (lagged by L tiles) by mul_{i-L} + store_{i-L}, so each
                    # HWDGE ring alternates ~1 MB reads and writes and the
                    # HBM sees mixed traffic instead of separate phases.
                    # Stores must follow their muls in PROGRAM order for the
                    # tile framework to emit the right RAW dependency.
                    L = interleave
                    mubs = []
                    for k, (ga, gw) in enumerate(granges):
                        mt, _ = mtiles[k]
                        gq = gw // mdiv
                        mub = pool.tile([P, gw], mybir.dt.uint16, tag=f"mu{k}")
                        for j in range(mdiv):
                            nc.vector.tensor_scalar(
                                out=mub[:, j * gq:(j + 1) * gq], in0=mt[:],
                                scalar1=shift * j, scalar2=fmask,
                                op0=mybir.AluOpType.logical_shift_right,
                                op1=mybir.AluOpType.bitwise_and)
                        mubs.append(mub)
                    xts = []

                    def mul_store(j):
                        k = tile2group[j]
                        ga = granges[k][0]
                        a, w = offs[j], sizes[j]
                        nc.vector.tensor_mul(
                            out=xts[j][:], in0=xts[j][:],
                            in1=mubs[k][:, a - ga: a - ga + w])
                        store_eng = getattr(
                            nc, store_qs[(j + 1) % len(store_qs)])
                        store_eng.dma_start(out=sl(o, a, w), in_=xts[j][:])

                    for i, (a, w) in enumerate(zip(offs, sizes)):
                        load_eng = getattr(nc, load_qs[i % len(load_qs)])
                        xt = pool.tile([P, w], mybir.dt.bfloat16,
                                       tag=f"xt{i}")
                        load_eng.dma_start(out=xt[:], in_=sl(x, a, w))
                        xts.append(xt)
                        if i >= L:
                            mul_store(i - L)
                    for j in range(len(sizes) - L, len(sizes)):
                        mul_store(j)
                    continue
                xts = []
                for i, (a, w) in enumerate(zip(offs, sizes)):
                    if i in swdge_tiles:
                        load_eng = nc.gpsimd       # 3rd channel: SWDGE
                    else:
                        load_eng = getattr(nc, load_qs[i % len(load_qs)])
                    xt = pool.tile([P, w], mybir.dt.bfloat16, tag=f"xt{i}")
                    load_eng.dma_start(out=xt[:], in_=sl(x, a, w))
                    xts.append(xt)
                if mode != "full":
                    # BW diagnostics, no compute.  "loads": loads + one
                    # token store.  "copy": loads + all stores (isolates
                    # the compute overhang when compared against "full").
                    store_tiles = [len(sizes) - 1] if mode == "loads" \
                        else list(range(len(sizes)))
                    if rev_store:
                        store_tiles = store_tiles[::-1]
                    for pos, i in enumerate(store_tiles):
                        store_eng = getattr(nc, store_qs[pos % len(store_qs)])
                        store_eng.dma_start(
                            out=sl(o, offs[i], sizes[i]), in_=xts[i][:])
                    continue
                # unpack each mask group to a full-width u16 multiplier tile
                # ({0,2} nibbles, or {0,1} bits with x pre-scaled by 2):
                # field j of word i covers group column j*gq + i
                mubs = []
                for k, (ga, gw) in enumerate(granges):
                    mt, _ = mtiles[k]
                    gq = gw // mdiv
                    mub = pool.tile([P, gw], mybir.dt.uint16, tag=f"mu{k}")
                    for j in range(mdiv):
                        nc.vector.tensor_scalar(
                            out=mub[:, j * gq:(j + 1) * gq], in0=mt[:],
                            scalar1=shift * j, scalar2=fmask,
                            op0=mybir.AluOpType.logical_shift_right,
                            op1=mybir.AluOpType.bitwise_and)
                    mubs.append(mub)
                for i, (a, w) in enumerate(zip(offs, sizes)):
                    k = tile2group[i]
                    ga = granges[k][0]
                    ce = nc.vector if engs[i] == 'v' else nc.gpsimd
                    ce.tensor_mul(out=xts[i][:], in0=xts[i][:],
                                  in1=mubs[k][:, a - ga: a - ga + w])
                order = list(range(len(sizes)))
                if pool_last:
                    # Pool muls finish after the load phase; storing their
                    # tiles last keeps the store rings from stalling on them
                    order = ([i for i in order if engs[i] == 'v']
                             + [i for i in order if engs[i] == 'p'])
                if rev_store:
                    order = order[::-1]
                for pos, i in enumerate(order):
                    if i in swdge_tiles:
                        store_eng = nc.gpsimd
                    else:
                        store_eng = getattr(nc, store_qs[pos % len(store_qs)])
                    store_eng.dma_start(
                        out=sl(o, offs[i], sizes[i]), in_=xts[i][:])
    nc.compile()
    return nc


def _mask_layout(mflat_global, mask_bits=False, sizes=None, mask_groups=None):
    """Host staging of the mask operand: from the (N_CORES*ELEMS,) flat
    {0,1} byte mask, produce the packed group-major u16 words build_nc's
    merged group loads expect.

    Per group [P, gw]: word i of the packed [P, gw/mdiv] group tile holds
    group column j*(gw/mdiv) + i in field j — value m*2 per nibble
    (mdiv=4), or m per bit (mdiv=16, x pre-scaled by 2 on host).
    """
    sizes = sizes or SIZES
    mask_groups = mask_groups or MASK_GROUPS
    mdiv, shift = (16, 1) if mask_bits else (4, 4)
    pre = 0 if mask_bits else 1           # {0,1} bits vs {0,2} nibbles
    offs = np.cumsum([0] + list(sizes))[:-1]
    gb = np.cumsum([0] + list(mask_groups))
    per_core = mflat_global.reshape(N_CORES, ELEMS_PER_CORE)
    out = np.empty((N_CORES, ELEMS_PER_CORE // mdiv), dtype=np.uint16)
    for c in range(N_CORES):
        t2d = [per_core[c, P * a: P * (a + w)].reshape(P, w)
               for a, w in zip(offs, sizes)]
        pos = 0
        for k in range(len(mask_groups)):
            g = np.concatenate(t2d[gb[k]:gb[k + 1]], axis=1).astype(np.uint16)
            gq = g.shape[1] // mdiv
            g3 = g.reshape(P, mdiv, gq)
            words = np.zeros((P, gq), dtype=np.uint16)
            for j in range(mdiv):
                words |= g3[:, j, :] << (shift * j + pre)
            w_flat = np.ascontiguousarray(words).ravel()
            out[c, pos:pos + w_flat.size] = w_flat
            pos += w_flat.size
    return out.reshape(-1)


def _build_runner(nc, n_cores):
    """Compile the SPMD module into a reusable shard_map-jitted callable.

    Same machinery as bass2jax.run_bass_via_pjrt, but the jitted function is
    built once and cached so repeated kernel() calls skip XLA re-tracing.
    Output-buffer donation is dropped: this kernel writes every output
    element, so zero-initialized outputs are unnecessary.
    """
    import jax
    from jax.sharding import Mesh, PartitionSpec, NamedSharding
    from jax.experimental.shard_map import shard_map
    from concourse.bass2jax import (
        _bass_exec_p,
        install_neuronx_cc_hook,
        partition_id_tensor,
    )

    install_neuronx_cc_hook()
    partition_name = nc.partition_id_tensor.name if nc.partition_id_tensor else None

    in_names, out_names, out_avals = [], [], []
    for alloc in nc.m.functions[0].allocations:
        if not isinstance(alloc, mybir.MemoryLocationSet):
            continue
        name = alloc.memorylocations[0].name
        if alloc.kind == "ExternalInput":
            if name != partition_name:
                in_names.append(name)
        elif alloc.kind == "ExternalOutput":
            out_names.append(name)
            out_avals.append(
                jax.core.ShapedArray(
                    tuple(alloc.tensor_shape), mybir.dt.np(alloc.dtype)
                )
            )
    n_params = len(in_names)
    all_in_names = list(in_names) + list(out_names)
    if partition_name is not None:
        all_in_names.append(partition_name)

    def _body(*args):
        operands = list(args)
        if partition_name is not None:
            operands.append(partition_id_tensor())
        outs = _bass_exec_p.bind(
            *operands,
            out_avals=tuple(out_avals),
            in_names=tuple(all_in_names),
            out_names=tuple(out_names),
            lowering_input_output_aliases=(),
            sim_require_finite=True,
            sim_require_nnan=True,
            nc=nc,
        )
        return tuple(outs)

    devices = jax.devices()[:n_cores]
    assert len(devices) == n_cores, (
        f"need {n_cores} devices, have {len(jax.devices())}"
    )
    mesh = Mesh(np.asarray(devices), ("core",))
    in_specs = (PartitionSpec("core"),) * (n_params + len(out_names))
    out_specs = (PartitionSpec("core"),) * len(out_names)
    fn = jax.jit(
        shard_map(
            _body, mesh=mesh, in_specs=in_specs, out_specs=out_specs,
            check_rep=False,
        ),
        keep_unused=True,
    )
    sharding = NamedSharding(mesh, PartitionSpec("core"))
    zeros = [
        np.zeros((n_cores * a.shape[0], *a.shape[1:]), a.dtype) for a in out_avals
    ]
    return fn, sharding, in_names, out_avals, zeros


_CACHE = {}


def _get_runner():
    if "runner" not in _CACHE:
        nc = build_nc()
        _CACHE["runner"] = _build_runner(nc, N_CORES)
    return _CACHE["runner"]


def kernel(x: np.ndarray, mask: np.ndarray, **_) -> np.ndarray:
    import jax
    import ml_dtypes

    x = np.ascontiguousarray(np.asarray(x), dtype=np.float32)
    mask = np.asarray(mask)
    if mask.dtype.itemsize != 1:
        mask = mask.astype(np.bool_)
    mask = np.ascontiguousarray(mask)
    assert x.shape == (B, C, FM, H, W), x.shape
    assert mask.shape == (B, C, FM, H, W), mask.shape

    fn, sharding, in_names, out_avals, zeros = _get_runner()
    # Flat layout: batch-sharding == contiguous row-blocks, and the kernel's
    # element order is plain C order, so x is a zero-copy reshape.  x
    # travels as bf16 (round-to-nearest); the mask travels nibble-packed
    # with the 1/(1-p)=2.0 dropout scale folded into the nibble values.
    xs = x * np.float32(2.0) if MASK_BITS else x   # {0,1} bit masks need 2x
    global_in = {
        "x": xs.astype(ml_dtypes.bfloat16).reshape(N_CORES * ELEMS_PER_CORE),
        "mask": _mask_layout(
            mask.view(np.uint8).reshape(N_CORES * ELEMS_PER_CORE),
            mask_bits=MASK_BITS),
    }
    if "zeros_dev" not in _CACHE:
        # Output buffers are fully overwritten by the kernel; stage once and
        # reuse across calls (not donated).
        _CACHE["zeros_dev"] = [jax.device_put(z, sharding) for z in zeros]
    args = [jax.device_put(global_in[n], sharding) for n in in_names]
    args += _CACHE["zeros_dev"]
    out = jax.block_until_ready(fn(*args))
    return np.asarray(out[0]).astype(np.float32).reshape(B, C, FM, H, W)
